# revision 7
# baseline (speedup 1.0000x reference)
"""Residual VQ (4-level, K=1024, D=512) Bass kernel for Trainium2.

Data-parallel over N across 8 NeuronCores; codebooks replicated. Per core,
per 128-row block, depth-first over the 4 levels:
  scores m2 = 2*r @ cb^T via 3-pass bf16-split matmuls on PE
  (rh*ch + rh*cl + rl*ch, fp32 PSUM accumulate ~= fp32 precision),
  s = m2 - ||cb||^2 on DVE, top-1 via DVE max/max_index,
  embed row-gather via indirect DMA, residual kept transposed
  (PE transpose of embeds, fused DVE subtract).
"""

import numpy as np
from contextlib import ExitStack

N, D, K, L = 65536, 512, 1024, 4
NCORES = 8
NSH = N // NCORES  # 8192 rows per core
P = 128
NT = NSH // P      # 64 blocks per core
DC = D // P        # 4 contraction chunks
KC = K // 512      # 2 moving chunks

_CACHE = {}


def _build_nc():
    import concourse.bass as bass
    import concourse.bacc as bacc
    import concourse.mybir as mybir
    import concourse.tile as tile
    from concourse.masks import make_identity

    f32 = mybir.dt.float32
    bf16 = mybir.dt.bfloat16
    u32 = mybir.dt.uint32

    nc = bacc.Bacc(None, target_bir_lowering=False)
    x = nc.declare_dram_parameter("x", [NSH, D], f32, isOutput=False)
    cbh2 = nc.declare_dram_parameter("cbh2", [L, D, K], bf16, isOutput=False)
    cbl2 = nc.declare_dram_parameter("cbl2", [L, D, K], bf16, isOutput=False)
    csq = nc.declare_dram_parameter("csq", [L, P, K], f32, isOutput=False)
    cb_dram = [
        nc.declare_dram_parameter(f"cb{l}", [K, D], f32, isOutput=False)
        for l in range(L)
    ]
    quant = nc.declare_dram_parameter("quant", [NSH, D], f32, isOutput=True)
    codes = nc.declare_dram_parameter("codes", [NSH, L], u32, isOutput=True)

    with tile.TileContext(nc) as tc, ExitStack() as ctx:
        consts = ctx.enter_context(tc.tile_pool(name="consts", bufs=1))
        ident = consts.tile([P, P], f32)
        make_identity(nc, ident[:])
        ch_sb = consts.tile([P, L, DC, K], bf16)
        nc.sync.dma_start(ch_sb[:], cbh2.rearrange("l (c p) k -> p l c k", p=P))
        cl_sb = consts.tile([P, L, DC, K], bf16)
        nc.sync.dma_start(cl_sb[:], cbl2.rearrange("l (c p) k -> p l c k", p=P))
        csq_sb = consts.tile([P, L, K], f32)
        nc.sync.dma_start(csq_sb[:], csq.rearrange("l p k -> p l k"))
        codes_all = consts.tile([P, NT, L], u32)

        xpool = ctx.enter_context(tc.tile_pool(name="x", bufs=3))
        rtpool = ctx.enter_context(tc.tile_pool(name="rt", bufs=2))
        rhpool = ctx.enter_context(tc.tile_pool(name="rh", bufs=2))
        spool = ctx.enter_context(tc.tile_pool(name="s", bufs=2))
        epool = ctx.enter_context(tc.tile_pool(name="emb", bufs=2))
        qpool = ctx.enter_context(tc.tile_pool(name="q", bufs=2))
        i8pool = ctx.enter_context(tc.tile_pool(name="i8", bufs=3))
        pspool = ctx.enter_context(tc.tile_pool(name="ps", bufs=2, space="PSUM"))
        ptpool = ctx.enter_context(tc.tile_pool(name="pt", bufs=2, space="PSUM"))

        for b in range(NT):
            x_t = xpool.tile([P, D], f32)
            nc.sync.dma_start(x_t[:], x[b * P:(b + 1) * P, :])
            rT = rtpool.tile([P, DC, P], f32)
            ptx = ptpool.tile([P, D], f32)
            for d in range(DC):
                nc.tensor.transpose(
                    ptx[:, d * P:(d + 1) * P], x_t[:, d * P:(d + 1) * P], ident[:]
                )
            nc.vector.tensor_copy(rT[:], ptx[:])
            q_t = qpool.tile([P, D], f32)
            for l in range(L):
                rh = rhpool.tile([P, DC, P], bf16)
                rl = rhpool.tile([P, DC, P], bf16)
                nc.vector.tensor_copy(rh[:], rT[:])
                nc.vector.tensor_sub(rl[:], rT[:], rh[:])
                ps = pspool.tile([P, K], f32)
                for kc in range(KC):
                    ksl = slice(kc * 512, (kc + 1) * 512)
                    for d in range(DC):
                        nc.tensor.matmul(
                            ps[:, ksl], lhsT=rh[:, d, :],
                            rhs=ch_sb[:, l, d, ksl],
                            start=(d == 0), stop=False,
                        )
                        nc.tensor.matmul(
                            ps[:, ksl], lhsT=rh[:, d, :],
                            rhs=cl_sb[:, l, d, ksl],
                            start=False, stop=False,
                        )
                        nc.tensor.matmul(
                            ps[:, ksl], lhsT=rl[:, d, :],
                            rhs=ch_sb[:, l, d, ksl],
                            start=False, stop=(d == DC - 1),
                        )
                s_t = spool.tile([P, K], f32)
                nc.vector.tensor_sub(s_t[:], ps[:], csq_sb[:, l, :])
                t8 = i8pool.tile([P, 8], f32)
                i8 = i8pool.tile([P, 8], u32)
                nc.vector.max(out=t8[:], in_=s_t[:])
                nc.vector.max_index(out=i8[:], in_max=t8[:], in_values=s_t[:])
                nc.vector.tensor_copy(codes_all[:, b, l:l + 1], i8[:, :1])
                emb = epool.tile([P, D], f32)
                nc.gpsimd.indirect_dma_start(
                    out=emb[:],
                    out_offset=None,
                    in_=cb_dram[l][:],
                    in_offset=bass.IndirectOffsetOnAxis(ap=i8[:, :1], axis=0),
                )
                if l == 0:
                    nc.gpsimd.tensor_copy(q_t[:], emb[:])
                else:
                    nc.gpsimd.tensor_add(q_t[:], q_t[:], emb[:])
                if l < L - 1:
                    pe_ = ptpool.tile([P, D], f32)
                    for d in range(DC):
                        nc.tensor.transpose(
                            pe_[:, d * P:(d + 1) * P], emb[:, d * P:(d + 1) * P],
                            ident[:],
                        )
                    nc.vector.tensor_sub(rT[:], rT[:], pe_[:])
            nc.sync.dma_start(quant[b * P:(b + 1) * P, :], q_t[:])
        nc.sync.dma_start(codes.rearrange("(b p) l -> p b l", p=P), codes_all[:])
    nc.compile()
    return nc


def _get_nc():
    if "nc" not in _CACHE:
        _CACHE["nc"] = _build_nc()
    return _CACHE["nc"]


def _host_inputs(inputs, codebooks):
    import ml_dtypes

    x = np.ascontiguousarray(inputs, dtype=np.float32)
    cbs = np.ascontiguousarray(codebooks, dtype=np.float32)
    c2 = np.ascontiguousarray(2.0 * cbs.transpose(0, 2, 1))            # [L, D, K]
    ch = c2.astype(ml_dtypes.bfloat16)
    cl = (c2 - ch.astype(np.float32)).astype(ml_dtypes.bfloat16)
    c_sq = (cbs.astype(np.float32) ** 2).sum(-1).astype(np.float32)    # [L, K]
    csq = np.ascontiguousarray(
        np.broadcast_to(c_sq[:, None, :], (L, P, K))
    ).astype(np.float32)
    common = {"cbh2": ch, "cbl2": cl, "csq": csq}
    for l in range(L):
        common[f"cb{l}"] = np.ascontiguousarray(cbs[l])
    in_maps = []
    for c in range(NCORES):
        m = dict(common)
        m["x"] = np.ascontiguousarray(x[c * NSH:(c + 1) * NSH])
        in_maps.append(m)
    return in_maps


def run_on_hw(inputs, codebooks, trace=False):
    from concourse.bass_utils import run_bass_kernel_spmd

    nc = _get_nc()
    in_maps = _host_inputs(inputs, codebooks)
    res = run_bass_kernel_spmd(nc, in_maps, list(range(NCORES)), trace=trace)
    quant = np.concatenate([r["quant"] for r in res.results], axis=0)
    codes = np.concatenate([r["codes"] for r in res.results], axis=0)
    return quant, codes.view(np.int32), res


def kernel(inputs, codebooks):
    quant, codes, _ = run_on_hw(inputs, codebooks, trace=False)
    return quant, codes
